# revision 32
# baseline (speedup 1.0000x reference)
"""BERT-CRF loss kernel for Trainium2 (8 NeuronCores, Bass/Tile).

Scaled-exp domain CRF forward with an exactly-32-row state per batch column:
  rows 0-30: P = exp(alpha[kept_tags] - t*MU)   (START tag dropped: provably 0)
  row 31:    omega = captured forward value (self-holding)

One step (t = 1..511):  state' = (W^T state) * F_t   where
  W[:31, :31] = exp(trans)[kept, kept]   (CRF transition mixing)
  W[:31, 31]  = 1, W[31, 31] = 1         (omega' = colsum(P) + omega)
  F_t rows 0-30 = exp(feat_t - MU)  (0 at the death step t==len: host scatter)
  F_t row 31    = theta_t = (t >= len)   (step fn; omega captures once because
                                          P dies at the death step, so colsum
                                          contributes only at t==len)

K=M=32 keeps every matmul in a single PE row/col group (one instruction), and
the 32-channel augmented feats keep all SBUF partition slices 32-aligned.
After a final virtual step 512 (captures len==512), forward[b] =
log(omega) + len[b]*MU. Gold score is pure gathers, done on host.
Validated: max |log| magnitude ~59 < 88 (fp32 safe) with MU=4.3.
"""

import os
import sys

import numpy as np

NUM_TAGS = 32
START = 30
STOP = 31
B = 1024
S = 512
NCORES = 8
BC = B // NCORES  # 128 batch per core
MU = 4.3
ST = 32  # state rows: 31 kept tags + omega
WIDTHS = (64, 64)  # per-chain batch columns (even, 4B-aligned offsets)
OFFS = (0, 64)
NCH = len(WIDTHS)
NEG = -1.0e9
KEPT = list(range(30)) + [31]  # all tags except START

for _p in ("/opt/trn_rl_repo", "/root/.axon_site/_ro/trn_rl_repo"):
    if os.path.isdir(_p) and _p not in sys.path:
        sys.path.append(_p)

_NC_CACHE = None
_LAST_RESULTS = None  # BassKernelResults of most recent device run (for test.py)


def _install_ntff_hook():
    """Shim antenv.axon_hooks (absent in this image) so trace=True works."""
    import types

    if "antenv.axon_hooks" in sys.modules:
        return
    mod = types.ModuleType("antenv.axon_hooks")
    mod._hook = None
    mod.set_axon_ntff_profile_hook = lambda h: setattr(mod, "_hook", h)
    mod.get_axon_ntff_profile_hook = lambda: mod._hook
    sys.modules["antenv.axon_hooks"] = mod
    try:
        import antenv

        antenv.axon_hooks = mod
    except ImportError:
        pass
    try:
        from trn_agent_boot.trn_boot import _ntff_profile_via_ctypes

        h = _ntff_profile_via_ctypes("/opt/axon/libaxon_pjrt.so")
        if h is not None:
            mod._hook = h
    except Exception:
        pass


def _build_bass():
    import concourse.bacc as bacc
    import concourse.tile as tile
    from concourse import mybir

    f32 = mybir.dt.float32
    bf16 = mybir.dt.bfloat16
    nc = bacc.Bacc(None)

    aug_d = nc.declare_dram_parameter("aug", [BC, S, ST], f32, isOutput=False)
    ident_d = nc.declare_dram_parameter("ident", [128, 128], bf16, isOutput=False)
    w_d = nc.declare_dram_parameter("w", [3 * ST, ST], bf16, isOutput=False)
    fin_d = nc.declare_dram_parameter("fin", [ST, BC], bf16, isOutput=False)
    esel_d = nc.declare_dram_parameter("esel", [3 * ST, 1], bf16, isOutput=False)
    fs_d = nc.declare_dram_parameter("fs", [1, BC], f32, isOutput=True)

    Exp = mybir.ActivationFunctionType.Exp

    with tile.TileContext(nc) as tc:
        with (
            tc.tile_pool(name="const", bufs=1) as const,
            tc.tile_pool(name="fsb", bufs=5) as fsb_pool,
            tc.tile_pool(name="fexp", bufs=5) as f_pool,
            tc.tile_pool(name="febm", bufs=3) as fe_pool,
            tc.tile_pool(name="pa", bufs=2) as pa_pool,
            tc.tile_pool(name="pb", bufs=2) as pb_pool,
            tc.tile_pool(name="pc", bufs=2) as pc_pool,
            tc.tile_pool(name="tp", bufs=3, space="PSUM") as tp_pool,
            tc.tile_pool(name="qa", bufs=1, space="PSUM") as qa_pool,
            tc.tile_pool(name="qb", bufs=1, space="PSUM") as qb_pool,
            tc.tile_pool(name="qc", bufs=1, space="PSUM") as qc_pool,
        ):
            w_sb = const.tile([3 * ST, ST], bf16)
            nc.sync.dma_start(out=w_sb, in_=w_d[:, :])
            ident_sb = const.tile([128, 128], bf16)
            nc.sync.dma_start(out=ident_sb, in_=ident_d[:, :])
            fin_sb = const.tile([ST, BC], bf16)
            nc.sync.dma_start(out=fin_sb, in_=fin_d[:, :])
            esel_sb = const.tile([3 * ST, 1], bf16)
            nc.sync.dma_start(out=esel_sb, in_=esel_d[:, :])
            biasmu_sb = const.tile([128, 1], f32)
            nc.vector.memset(biasmu_sb, -MU)
            fs_sb = const.tile([1, BC], f32)

            # pre-touch DMA'd constants on PE so real PE ops carry <=1 new wait
            warm = qa_pool.tile([ST, BC], f32, tag="qa")
            warmt = tp_pool.tile([128, 512], bf16, tag="tp", name="tp")
            nc.tensor.transpose(warmt[0:ST, 0:128], ident_sb[:, 0:ST], ident_sb)
            nc.tensor.matmul(
                warm[:, 0:ST], w_sb[0:ST, :], w_sb[0:ST, 0:ST],
                start=True, stop=True,
            )
            nc.tensor.matmul(warm, w_sb[0:ST, :], fin_sb, start=True, stop=True)
            nc.tensor.matmul(
                warm[:, 0:1], w_sb[0:ST, :], esel_sb[0:ST, :], start=True, stop=True
            )

            # chain c's state lives at partition base 32*c (PE row group c)
            # so the per-step matmuls run concurrently in separate PE
            # sub-arrays; W/esel are replicated per row group.
            q_pools = (qa_pool, qb_pool, qc_pool)
            p_pools = (pa_pool, pb_pool, pc_pool)
            states = [None, None, None]  # (ap, base) per chain
            bases = [0, 0, 0]

            def step(f_row, f_cols):
                """state' = (W^T state) * F for all chains."""
                for c in range(NCH):
                    w0 = WIDTHS[c]
                    q = q_pools[c].tile([ST, w0], f32, tag="q", name=f"q{c}")
                    nc.tensor.matmul(
                        q, w_sb[0:ST, :], states[c], start=True, stop=True
                    )
                    p_n = p_pools[c].tile([ST, w0], bf16, tag="p", name=f"p{c}")
                    nc.vector.tensor_mul(
                        p_n, q, f_row[:, f_cols[c] : f_cols[c] + w0]
                    )
                    states[c] = p_n
                    bases[c] = 0

            def stage(g):
                """DMA + exp (batch-major) + bf16 transpose + copy one group."""
                fsb = fsb_pool.tile([128, 16 * ST], f32, tag="fsb", name="fsb")
                nc.sync.dma_start(
                    out=fsb,
                    in_=aug_d[:, 16 * g : 16 * (g + 1), :].rearrange(
                        "b t j -> b (t j)"
                    ),
                )
                fe = fe_pool.tile([128, 512], bf16, tag="fe", name="fe")
                nc.scalar.activation(fe, fsb, Exp, bias=biasmu_sb, scale=1.0)
                tp = tp_pool.tile([128, 512], bf16, tag="tp", name="tp")
                for k in range(4):  # bf16 single-pass PE transposes
                    nc.tensor.transpose(
                        tp[:, 128 * k : 128 * (k + 1)],
                        fe[:, 128 * k : 128 * (k + 1)],
                        ident_sb,
                    )
                f_tile = f_pool.tile([128, 512], bf16, tag="f", name="f")
                nc.scalar.copy(f_tile, tp)
                return f_tile

            NG = S // 16
            f_tiles = {0: stage(0), 1: stage(1), 2: stage(2)}
            for g in range(NG):  # staging runs 3 groups ahead of the steps
                nxt = g + 3
                if nxt < NG:
                    fsb_n = fsb_pool.tile([128, 16 * ST], f32, tag="fsb", name="fsb")
                    nc.sync.dma_start(
                        out=fsb_n,
                        in_=aug_d[:, 16 * nxt : 16 * (nxt + 1), :].rearrange(
                            "b t j -> b (t j)"
                        ),
                    )
                    fe_n = fe_pool.tile([128, 512], bf16, tag="fe", name="fe")
                    nc.scalar.activation(fe_n, fsb_n, Exp, bias=biasmu_sb, scale=1.0)
                    tp_n = tp_pool.tile([128, 512], bf16, tag="tp", name="tp")
                f_tile = f_tiles.pop(g)
                for s in range(16):
                    t = 16 * g + s
                    k, sub = s // 4, s % 4
                    r = 32 * sub
                    c = 128 * k
                    if nxt < NG and sub == 0:
                        # one of group g+3's bf16 transposes per quad
                        nc.tensor.transpose(
                            tp_n[:, 128 * k : 128 * (k + 1)],
                            fe_n[:, 128 * k : 128 * (k + 1)],
                            ident_sb,
                        )
                    if t == 0:
                        for ci in range(NCH):
                            states[ci] = f_tile[0:ST, OFFS[ci] : OFFS[ci] + WIDTHS[ci]]
                            bases[ci] = 0
                        continue
                    step(f_tile[r : r + ST, c : c + BC], OFFS)
                if nxt < NG:
                    f_new = f_pool.tile([128, 512], bf16, tag="f", name="f")
                    nc.scalar.copy(f_new, tp_n)
                    f_tiles[nxt] = f_new

            # virtual step 512: capture len==512 columns via fin
            step(fin_sb, OFFS)

            # fs = omega row = esel^T state
            fs_ps = qa_pool.tile([1, BC], f32, tag="fsps", bufs=1)
            for c in range(NCH):
                nc.tensor.matmul(
                    fs_ps[:, OFFS[c] : OFFS[c] + WIDTHS[c]],
                    esel_sb[bases[c] : bases[c] + ST, :],
                    states[c],
                    start=True,
                    stop=True,
                )
            nc.vector.tensor_copy(fs_sb, fs_ps)
            nc.sync.dma_start(out=fs_d[:, :], in_=fs_sb)

    if not nc.is_finalized():
        nc.finalize()
    return nc


def _gold_score(feats, labels, lengths, trans):
    pos = np.arange(S)[None, :]
    valid = pos < lengths[:, None]
    emit = np.take_along_axis(feats, labels[:, :, None], axis=2)[:, :, 0]
    emit_sum = np.where(valid, emit, 0.0).sum(axis=1)
    start_sc = trans[START, labels[:, 0]]
    pair = trans[labels[:, :-1], labels[:, 1:]]
    pair_sum = np.where(valid[:, 1:], pair, 0.0).sum(axis=1)
    last_tag = np.take_along_axis(labels, (lengths - 1)[:, None], axis=1)[:, 0]
    stop_sc = trans[last_tag, STOP]
    return emit_sum + start_sc + pair_sum + stop_sc


def kernel(feats, labels, lengths, transitions):
    global _NC_CACHE, _LAST_RESULTS
    import ml_dtypes
    from concourse.bass_utils import run_bass_kernel_spmd

    feats = np.asarray(feats, dtype=np.float32)
    labels = np.asarray(labels, dtype=np.int64)
    lengths = np.asarray(lengths, dtype=np.int64)
    trans = np.asarray(transitions, dtype=np.float32)

    if _NC_CACHE is None:
        _NC_CACHE = _build_bass()
    nc = _NC_CACHE

    # augmented feats: [B, S, 32] = [feats[kept] | theta_log]
    aug = np.empty((B, S, ST), np.float32)
    aug[:, :, :31] = feats[:, :, KEPT]
    rows = lengths <= S - 1
    aug[np.arange(B)[rows], lengths[rows], :31] = NEG  # kill column at t==len
    aug[:, :, 31] = np.where(
        np.arange(S)[None, :] >= lengths[:, None], MU, NEG
    )  # theta step fn (0 at t=0 since len>=1)
    aug[:, 0, :31] += trans[START, KEPT]  # fold start-transition bias into t=0


    w = np.zeros((ST, ST), np.float32)
    w[:31, :31] = np.exp(trans)[np.ix_(KEPT, KEPT)]
    w[:31, 31] = 1.0  # omega' = colsum(P) + omega
    w[31, 31] = 1.0
    w = np.concatenate([w, w, w], axis=0).astype(ml_dtypes.bfloat16)

    ident = np.eye(128, dtype=ml_dtypes.bfloat16)

    fin_full = np.zeros((ST, B), ml_dtypes.bfloat16)
    fin_full[31] = 1.0  # virtual step 512: theta=1 everywhere, tags die

    esel = np.zeros((3 * ST, 1), ml_dtypes.bfloat16)
    esel[31, 0] = 1.0
    esel[63, 0] = 1.0
    esel[95, 0] = 1.0

    in_maps = []
    for c in range(NCORES):
        sl = slice(c * BC, (c + 1) * BC)
        in_maps.append(
            {
                "aug": aug[sl],
                "w": w,
                "fin": np.ascontiguousarray(fin_full[:, sl]),
                "esel": esel,
                "ident": ident,
            }
        )

    trace = bool(int(os.environ.get("BASS_KERNEL_TRACE", "0")))
    kw = {}
    if trace:
        import concourse.bass_utils as _bu

        _install_ntff_hook()
        _bu.upload_artifacts = lambda tmpdir: "local://" + tmpdir
        import tempfile

        root = os.environ.get("BASS_TRACE_DIR", "/tmp/bass_trace")
        os.makedirs(root, exist_ok=True)
        tdir = tempfile.mkdtemp(dir=root)
        kw = {"tmpdir": tdir}
    res = run_bass_kernel_spmd(nc, in_maps, list(range(NCORES)), trace=trace, **kw)
    _LAST_RESULTS = res

    fs = np.concatenate([res.results[c]["fs"][0] for c in range(NCORES)])  # [B]
    forward = np.log(fs.astype(np.float64)) + lengths * MU
    gold = _gold_score(feats, labels, lengths, trans).astype(np.float64)
    loss = np.sum(forward - gold) / B
    return np.asarray(loss, dtype=np.float32)
